# revision 39
# baseline (speedup 1.0000x reference)
"""Trainium2 Bass kernel for nn_CNNConcatLinear (B=1024, N=24, PD=2, C=512).

Strategy: pure data-parallel over batch (128 per core x 8 cores).
Per core, channels-on-partitions layouts:

  phase A: new_ctx = context + (sum_j e_j t_j)/(sum_j e_j)  (the two 1x1
           convs + softmax collapse into a 3x3 system folded on host).
  phase B: all ConcatSquashLinear gates/hyper-biases as [feature, batch]
           bf16 matmuls from new_ctx^T.  Hyper-bias/gate ratios (h/g) are
           transposed once per core so later layers can fold "+ h" into
           their matmuls via batch-indicator contraction rows.
  phase C: pipeline over batch chunks of 16:
           c1 (x-rows + b-indicator fold rows) -> gated X in fp8 (x16) ->
           6 convs as fp8 DoubleRow matmuls (2 ci-chunks per pass) ->
           c3/c4 as fp8 DoubleRow with bias+hyper folded as extra
           matmul rows (n-indicator rows carry c3bias incl. positional
           encoding pushed through the convs on host) -> cl in f32r.

All epilogues are a single DVE op: out = (psum * 2^-k) * gate.
"""

import math
import os

import numpy as np

B, N, PD, C = 1024, 24, 2, 512
F = 2 * C
NCORES = 8
BLOC = B // NCORES          # 128 batch per core
BC = 16                     # batch chunk
NBC = BLOC // BC            # 8 chunks
PADL = 5
NW = N + 2 * PADL           # 34 (padded X width)
FREE = BC * N               # 384

# quantization scales (powers of 2)
SX = 16.0                   # X fp8 scale
SW = 128.0                  # conv weight fp8 scale
SY = 16.0                   # Y fp8 scale
S3W = 128.0                 # c3 weight fp8 scale
ST3 = 16.0                  # T3 fp8 scale
S4W = 128.0                 # c4 weight fp8 scale
SGW = 64.0                  # gate weight fp8 scale
SCX = 16.0                  # new_ctx fp8 scale
PS3 = SY * S3W              # c3 psum scale (2048)
PS4 = ST3 * S4W             # c4 psum scale (2048)

TAPS = {
    0: [0], 1: [0], 2: [0], 3: [0],
    4: [0, -1, 1], 5: [0, -1, 1],
    6: [0, -1, 1, -2, 2, -3, 3],
    7: [0, -1, 1, -2, 2, -3, 3, -4, 4, -5, 5],
}
BLK = {}
for _co in range(8):
    for _d in TAPS[_co]:
        BLK[(_co, _d)] = len(BLK)
NBLK = len(BLK)             # 28

LAST_RESULTS = None         # BassKernelResults from the most recent run


def _pe_table():
    pos = np.arange(N, dtype=np.float32)[:, None]
    div = np.exp(np.arange(0, F, 2, dtype=np.float32) * (-np.log(10000.0) / F))
    pe = np.zeros((N, F), dtype=np.float32)
    pe[:, 0::2] = np.sin(pos * div)
    pe[:, 1::2] = np.cos(pos * div)
    return pe


def _f32(a):
    return np.ascontiguousarray(np.asarray(a, dtype=np.float32))


def _build(host, num_devices=NCORES):
    import concourse.bass as bass
    import concourse.mybir as mybir
    import concourse.tile as tile
    from concourse import bacc
    from concourse.masks import make_identity

    f32 = mybir.dt.float32
    F8 = mybir.dt.float8e4
    BF = mybir.dt.bfloat16
    DT = mybir.dt.float32r
    DR = mybir.MatmulPerfMode.DoubleRow
    AluOp = mybir.AluOpType
    Act = mybir.ActivationFunctionType

    M3, v3, s3 = host["M3"], host["v3"], host["s3"]

    nc = bacc.Bacc("TRN2", target_bir_lowering=False, debug=False,
                   num_devices=num_devices)

    def din(name, shape, dt=f32):
        return nc.dram_tensor(name, list(shape), dt, kind="ExternalInput").ap()

    ctx_d = din("ctx", [BLOC, C])
    beta_d = din("betav", [BLOC, 3])
    xtc_d = din("xtc", [NBC, 3, F + FREE], DT)
    wg_d = din("wg", [29, 128, 4, 128], BF)
    sm32_d = din("sm32", [128, 32])
    smbf_d = din("smbf", [24, 896], BF)
    convt_d = din("convt", [4, 128, NBLK * 2 * 128], F8)
    c3w8_d = din("c3w8", [128, 4, 2, C], F8)
    c4w8_d = din("c4w8", [128, 2, 2, 256], F8)
    clwt_d = din("clwt", [128, 2, PD], DT)
    indb_d = din("indb", [128, NBC, BC * N], F8)
    out_d = nc.dram_tensor("out", [BLOC * N, PD], f32, kind="ExternalOutput").ap()
    DEBUG = bool(int(os.environ.get("KERNEL_DEBUG", "0")))
    if DEBUG:
        dbg_nctx = nc.dram_tensor("dbg_nctx", [128, C], f32, kind="ExternalOutput").ap()
        dbg_g = nc.dram_tensor("dbg_g", [128, 2048], f32, kind="ExternalOutput").ap()
        dbg_y = nc.dram_tensor("dbg_y", [8, 128, BC, N], f32, kind="ExternalOutput").ap()
        dbg_t3 = nc.dram_tensor("dbg_t3", [4, 128, BC, N], f32, kind="ExternalOutput").ap()

    with tile.TileContext(nc) as tc:
        import contextlib
        est = contextlib.ExitStack()
        with est:
            wp = est.enter_context(tc.tile_pool(name="wp", bufs=1))
            gout = est.enter_context(tc.tile_pool(name="gout", bufs=1))

            # ---------- persistent small tiles + their DMAs ----------
            beta_t0 = wp.tile([128, 3], f32, tag="beta0")
            nc.sync.dma_start(beta_t0[:], beta_d[:])
            ctx_t0 = wp.tile([128, C], f32, tag="ctx0")
            nc.sync.dma_start(ctx_t0[:], ctx_d[:])
            # merged small tensors: 2 DMAs instead of 6
            sm32_s = wp.tile([128, 32], f32, tag="sm32")
            nc.sync.dma_start(sm32_s[:], sm32_d[:])
            smbf_s = wp.tile([24, 896], BF, tag="smbf")
            nc.sync.dma_start(smbf_s[:], smbf_d[:])
            gbias_s = sm32_s[:, 0:29]
            c4bx_s = sm32_s[:, 29:31]
            clb_s = sm32_s[0:PD, 31:32]
            c3bT_s = smbf_s[:, 0:512].rearrange("p (m o) -> p m o", m=4)
            indn_s = smbf_s[:, 512:896].rearrange("p (b n) -> p b n", b=BC)
            ident = wp.tile([128, 128], f32, tag="ident")
            make_identity(nc, ident[:])
            # probes: stall the Pool/ACT DGE streams until ctx+beta landed so
            # their bulk transfers can't jump ahead on the DMA engines
            probe = wp.tile([1, 2], f32, tag="probe")
            nc.gpsimd.tensor_copy(probe[0:1, 0:1], ctx_t0[0:1, 0:1])
            nc.scalar.copy(probe[0:1, 1:2], ctx_t0[0:1, 0:1])
            indb_s = wp.tile([128, NBC, BC * N], F8, tag="indb")
            wg_all = wp.tile([128, 29, 4, 128], BF, tag="wg")
            _wb = [0, 8, 16, 24, 29]
            for _cw in range(4):
                _clo, _chi = _wb[_cw], _wb[_cw + 1]
                nc.scalar.dma_start(
                    wg_all[:, _clo:_chi],
                    wg_d[_clo:_chi].rearrange("c p k o -> p c k o"))
            xt_all = wp.tile([3, NBC, F + FREE], DT, tag="xt")
            nc.sync.dma_start(xt_all[:, 0], xtc_d[0])

            # gate/hyper output tiles [feature_part, chunk*128 + b]
            g1b1_s = gout.tile([128, 2048], f32, tag="g1b1")
            g3h3_s = gout.tile([128, 1024], f32, tag="g3h3")
            g4h4_s = gout.tile([128, 512], f32, tag="g4h4")
            gl_s = gout.tile([PD, 128], f32, tag="gl")
            hl_s = gout.tile([PD, 128], f32, tag="hl")
            # transposed hyper/gate folds
            b1gT_s = gout.tile([128, 8, 128], BF, tag="b1gT")
            h3gT_s = gout.tile([128, 4, 128], BF, tag="h3gT")
            h4gT_s = gout.tile([128, 2, 128], BF, tag="h4gT")

            # ---------- heavy weight tiles ----------
            convw_s = []
            for pc in range(4):
                t = wp.tile([128, NBLK, 2, 128], F8, tag=f"convw{pc}")
                convw_s.append(t)
            c3w_all = wp.tile([128, 4, 2, C], F8, tag="c3w")
            c4w_all = wp.tile([128, 2, 2, 256], F8, tag="c4w")
            clw_all = wp.tile([128, 2, PD], DT, tag="clw")

            def emit_bulk_dmas():
                nc.gpsimd.dma_start(indb_s[:], indb_d[:])
                for pc in range(4):
                    nc.gpsimd.dma_start(
                        convw_s[pc][:],
                        convt_d[pc].rearrange(
                            "p (blk j o) -> p blk j o", blk=NBLK, j=2, o=128))
                for _bc in range(1, NBC):
                    nc.gpsimd.dma_start(xt_all[:, _bc], xtc_d[_bc])
                nc.gpsimd.dma_start(c3w_all[:], c3w8_d[:])
                nc.gpsimd.dma_start(c4w_all[:], c4w8_d[:])
                nc.gpsimd.dma_start(clw_all[:], clwt_d[:])

            # ---------- phase-C SBUF pools (open early so c1(0) can run in B)
            xp = est.enter_context(tc.tile_pool(name="xp", bufs=1))

            def bcast(ap_2d, np_=N):
                return ap_2d.unsqueeze(2).broadcast_to(
                    [ap_2d.shape[0], BC, np_])

            X_gen = []
            for gen in range(2):
                gtiles = []
                for pc in range(4):
                    Xf = xp.tile([128, 2, BC, NW], F8, tag=f"x{gen}_{pc}")
                    gtiles.append(Xf)
                X_gen.append(gtiles)

            def _c1(bc, pspool, pstag="c1", psbufs=None):
                cs = bc * BC
                X_t = X_gen[bc % 2]
                xt_c = xt_all[:, bc, :]
                for fc in range(8):
                    ps1 = pspool.tile([128, BC, N], f32, tag=pstag,
                                      bufs=psbufs)
                    nc.tensor.matmul(
                        ps1[:], xt_c[:, fc * 128:(fc + 1) * 128],
                        xt_c[:, F:], start=True, stop=False)
                    nc.tensor.matmul(
                        ps1[:], b1gT_s[:, fc, :],
                        indb_s[:, bc, :].rearrange("p (b n) -> p b n", b=BC),
                        start=False, stop=True)
                    xi = X_t[fc // 2][:, fc % 2, :, PADL:PADL + N]
                    g1 = bcast(g1b1_s[:, fc * 128 + cs:fc * 128 + cs + BC])
                    nc.vector.tensor_mul(xi, ps1[:], g1)

            # ---------- phase A: new_ctx (pipelined in 4 col blocks) ----
            with tc.tile_pool(name="pa", bufs=1) as pap:
                ctx_t = ctx_t0
                bvec = beta_t0          # [128, 3] = (beta, sin b, cos b)

                u = pap.tile([128, 3], f32, tag="u")
                for j in range(3):
                    uj = u[:, j:j + 1]
                    nc.vector.tensor_scalar(uj, bvec[:, 0:1], float(M3[j, 0]),
                                            float(v3[j]), AluOp.mult, AluOp.add)
                    nc.vector.scalar_tensor_tensor(uj, bvec[:, 1:2],
                                                   float(M3[j, 1]),
                                                   uj, AluOp.mult, AluOp.add)
                    nc.vector.scalar_tensor_tensor(uj, bvec[:, 2:3],
                                                   float(M3[j, 2]),
                                                   uj, AluOp.mult, AluOp.add)

                emit_bulk_dmas()
                psa_cm = tc.tile_pool(name="ps_a", bufs=1, space="PSUM")
                psa = psa_cm.__enter__()
                nctxT = wp.tile([128, C], BF, tag="nctxT")
                for kb in range(4):
                    sl = slice(kb * 128, (kb + 1) * 128)
                    z = pap.tile([128, 128], f32, tag="z", bufs=2)
                    num = pap.tile([128, 128], f32, tag="num", bufs=2)
                    for j in range(3):
                        ej = psa.tile([128, 128], f32, tag="ej", bufs=2)
                        nc.scalar.activation(ej[:], ctx_t[:, sl], Act.Exp,
                                             bias=u[:, j:j + 1],
                                             scale=float(s3[j]))
                        if j == 0:
                            nc.vector.tensor_copy(z[:], ej[:])
                            nc.vector.tensor_scalar(num[:], ej[:],
                                                    bvec[:, 0:1], None,
                                                    AluOp.mult)
                        else:
                            nc.vector.tensor_add(z[:], z[:], ej[:])
                            nc.vector.scalar_tensor_tensor(num[:], ej[:],
                                                           bvec[:, j:j + 1],
                                                           num[:], AluOp.mult,
                                                           AluOp.add)
                    nc.vector.reciprocal(z[:], z[:])
                    nc.vector.tensor_mul(num[:], num[:], z[:])
                    nc.vector.tensor_add(ctx_t[:, sl], ctx_t[:, sl], num[:])
                    pst = psa.tile([128, 128], f32, tag="tr", bufs=2)
                    nc.tensor.transpose(pst[:], ctx_t[:, sl], ident[:])
                    nc.scalar.copy(nctxT[:, sl], pst[:])
                nctx = ctx_t
                if DEBUG:
                    nc.sync.dma_start(dbg_nctx[:], nctx[:])
                psa_cm.__exit__(None, None, None)

            # X pad memsets (DVE is idle during the gate matmuls)
            for gen in range(2):
                for pc in range(4):
                    Xf = X_gen[gen][pc]
                    nc.vector.memset(Xf[:, :, :, 0:PADL], 0.0)
                    nc.vector.memset(Xf[:, :, :, PADL + N:NW], 0.0)

            # ---------- phase B: gates ----------
            with (
                tc.tile_pool(name="ps_b", bufs=8, space="PSUM") as pbp,
                tc.tile_pool(name="gprep", bufs=1) as gpp,
            ):
                def gdst(c):
                    # c1 hyper rows (c 8..15) are pre-scaled by SX on host
                    if c < 8:
                        return g1b1_s[:, c * 128:(c + 1) * 128], True
                    if c < 16:
                        return g1b1_s[:, 1024 + (c - 8) * 128:1024 + (c - 7) * 128], False
                    if c < 20:
                        return g3h3_s[:, (c - 16) * 128:(c - 15) * 128], True
                    if c < 24:
                        return g3h3_s[:, 512 + (c - 20) * 128:512 + (c - 19) * 128], False
                    if c < 26:
                        return g4h4_s[:, (c - 24) * 128:(c - 23) * 128], True
                    return g4h4_s[:, 256 + (c - 26) * 128:256 + (c - 25) * 128], False

                def fold_one(src_g, src_h, psc_scale, dstT, m, extra=None):
                    # dstT[:, m, :] = transpose of psc_scale * src_h / src_g
                    # (one 128-col chunk; spread across engines)
                    r_t = gpp.tile([128, 128], f32, tag="r1", bufs=4)
                    nc.vector.reciprocal(r_t[:], src_g)
                    hg = gpp.tile([128, 128], f32, tag="hg", bufs=4)
                    nc.vector.scalar_tensor_tensor(hg[:], src_h, psc_scale,
                                                   r_t[:], AluOp.mult,
                                                   AluOp.mult)
                    if extra is not None:
                        nc.vector.tensor_scalar(hg[:], hg[:], extra, None,
                                                AluOp.add)
                    pst = pbp.tile([128, 128], f32, tag="gtr", bufs=2)
                    nc.tensor.transpose(pst[:], hg[:], ident[:])
                    nc.vector.tensor_copy(dstT[:, m, :], pst[:])

                # one psum bank per gate chunk so accumulation groups of
                # consecutive chunks never share has_written state
                for c in range(29):
                    gp_bank = pbp.tile([128, 2, 128], f32, tag="gps", bufs=4)
                    gw_t = wg_all[:, c]
                    for k in range(4):
                        rhs = nctxT[:, k * 128:(k + 1) * 128]
                        if c == 28:
                            nc.tensor.matmul(gp_bank[0:2, 0, :],
                                             gw_t[:, k, 0:2], rhs,
                                             start=(k == 0), stop=False)
                            nc.tensor.matmul(gp_bank[0:2, 1, :],
                                             gw_t[:, k, 2:4], rhs,
                                             start=False, stop=(k == 3))
                        else:
                            nc.tensor.matmul(gp_bank[:, 0, :],
                                             gw_t[:, k], rhs,
                                             start=(k == 0), stop=(k == 3))
                    if c == 28:
                        nc.scalar.activation(gl_s[:], gp_bank[0:2, 0, :],
                                             Act.Sigmoid,
                                             bias=gbias_s[0:2, 28:29])
                        nc.scalar.copy(hl_s[:], gp_bank[0:2, 1, :])
                    else:
                        dst, is_g = gdst(c)
                        if is_g:
                            nc.scalar.activation(dst, gp_bank[:, 0, :],
                                                 Act.Sigmoid,
                                                 bias=gbias_s[:, c:c + 1])
                        else:
                            nc.scalar.copy(dst, gp_bank[:, 0, :])
                    # incremental hyper/gate fold transposes
                    if 8 <= c < 16:
                        fc = c - 8
                        fold_one(g1b1_s[:, fc * 128:(fc + 1) * 128],
                                 g1b1_s[:, 1024 + fc * 128:1024 + (fc + 1) * 128],
                                 1.0, b1gT_s, fc)
                    elif 20 <= c < 24:
                        m = c - 20
                        fold_one(g3h3_s[:, m * 128:(m + 1) * 128],
                                 g3h3_s[:, 512 + m * 128:512 + (m + 1) * 128],
                                 PS3, h3gT_s, m)
                    elif 26 <= c < 28:
                        m2 = c - 26
                        fold_one(g4h4_s[:, m2 * 128:(m2 + 1) * 128],
                                 g4h4_s[:, 256 + m2 * 128:256 + (m2 + 1) * 128],
                                 PS4, h4gT_s, m2, extra=c4bx_s[:, m2:m2 + 1])
                    if c == 15:
                        _c1(0, pbp, "gps2", psbufs=2)

                if DEBUG:
                    nc.sync.dma_start(dbg_g[:], g1b1_s[:])

            clw_s = [clw_all[:, k, :] for k in range(2)]

            # ---------- phase C ----------
            with (
                tc.tile_pool(name="yp", bufs=8) as yp,
                tc.tile_pool(name="t3p", bufs=4) as t3p,
                tc.tile_pool(name="obp", bufs=3) as obp,
                tc.tile_pool(name="ps_c1", bufs=1, space="PSUM") as ps_c1,
                tc.tile_pool(name="ps_cv", bufs=2, space="PSUM") as ps_cv,
                tc.tile_pool(name="ps_c3", bufs=2, space="PSUM") as ps_c3,
                tc.tile_pool(name="ps_ms", bufs=3, space="PSUM") as ps_ms,
            ):
                def _phase_c():
                  for bc in range(NBC):
                    cs = bc * BC
                    X_t = X_gen[bc % 2]

                    # --- convs (x16 fp8; bias + pe folded into c3bT)
                    Y_t = []
                    for co in range(0, 8, 2):
                        if co == 4 and bc + 1 < NBC:
                            _c1(bc + 1, ps_c1)
                        Yc = yp.tile([128, 2, BC, N], F8, tag="y")
                        for j in range(2):
                            cj = co + j
                            psc = ps_cv.tile([128, BC, N], f32, tag="conv")
                            mms = [(d, pc) for d in TAPS[cj] for pc in range(4)]
                            for i, (d, pc) in enumerate(mms):
                                nc.tensor.matmul(
                                    psc[:],
                                    convw_s[pc][:, BLK[(cj, d)], :, :],
                                    X_t[pc][:, :, :, PADL + d:PADL + d + N],
                                    start=(i == 0), stop=(i == len(mms) - 1),
                                    perf_mode=DR)
                            nc.scalar.mul(Yc[:, j], psc[:], SY / (SX * SW))
                            if DEBUG and bc == 0:
                                nc.sync.dma_start(dbg_y[cj], psc[:])
                        Y_t.append(Yc)

                    # --- c3 (DR over 4 Y pairs + bias/hyper fold rows)
                    T3_t = []
                    for m2 in range(2):
                        T3m = t3p.tile([128, 2, BC, N], F8, tag="t3")
                        for jm in range(2):
                            m = m2 * 2 + jm
                            ps3 = ps_c3.tile([128, BC, N], f32, tag="c3")
                            for pc in range(4):
                                nc.tensor.matmul(
                                    ps3[:],
                                    c3w_all[:, pc, :, m * 128:(m + 1) * 128],
                                    Y_t[pc][:], start=(pc == 0), stop=False,
                                    perf_mode=DR)
                            nc.tensor.matmul(
                                ps3[:], c3bT_s[:, m, :], indn_s,
                                start=False, stop=False)
                            nc.tensor.matmul(
                                ps3[:], h3gT_s[:, m, :],
                                indb_s[:, bc, :].rearrange("p (b n) -> p b n", b=BC),
                                start=False, stop=True)
                            g3 = bcast(g3h3_s[:, m * 128 + cs:m * 128 + cs + BC])
                            nc.vector.scalar_tensor_tensor(
                                T3m[:, jm], ps3[:], ST3 / PS3, g3,
                                AluOp.mult, AluOp.mult)
                            if DEBUG and bc == 0:
                                nc.sync.dma_start(dbg_t3[m], ps3[:])
                        T3_t.append(T3m)

                    # --- c4 (DR over 2 T3 pairs + hyper fold rows)
                    T4_t = []
                    for m2 in range(2):
                        ps4 = ps_ms.tile([128, BC, N], f32, tag="ms")
                        for pc in range(2):
                            nc.tensor.matmul(
                                ps4[:],
                                c4w_all[:, pc, :, m2 * 128:(m2 + 1) * 128],
                                T3_t[pc][:], start=(pc == 0), stop=False,
                                perf_mode=DR)
                        nc.tensor.matmul(
                            ps4[:], h4gT_s[:, m2, :],
                            indb_s[:, bc, :].rearrange("p (b n) -> p b n", b=BC),
                            start=False, stop=True)
                        T4m = t3p.tile([128, BC, N], DT, tag="t4")
                        g4 = bcast(g4h4_s[:, m2 * 128 + cs:m2 * 128 + cs + BC])
                        nc.vector.scalar_tensor_tensor(
                            T4m[:], ps4[:], 1.0 / PS4, g4,
                            AluOp.mult, AluOp.mult)
                        T4_t.append(T4m)

                    # --- cl
                    psl_full = ps_ms.tile([128, BC, N], f32, tag="ms")
                    psl = psl_full[0:PD]
                    for k in range(2):
                        nc.tensor.matmul(psl[:], clw_s[k], T4_t[k][:],
                                         start=(k == 0), stop=(k == 1))
                    OF_full = t3p.tile([128, BC, N], f32, tag="of")
                    OF = OF_full[0:PD]
                    gl = gl_s[:, cs:cs + BC].unsqueeze(2).broadcast_to([PD, BC, N])
                    hl = hl_s[:, cs:cs + BC].unsqueeze(2).broadcast_to([PD, BC, N])
                    nc.vector.scalar_tensor_tensor(OF[:], psl[:], clb_s[:], gl,
                                                   AluOp.add, AluOp.mult)
                    nc.vector.tensor_add(OF[:], OF[:], hl)

                    # --- transpose [2, 384] -> [384, 2] in 128-blocks, DMA out
                    OFf = OF[:].rearrange("p b n -> p (b n)")
                    osb = obp.tile([128, 3, PD], f32, tag="ob")
                    for blk in range(3):
                        ptr_full = ps_ms.tile([128, BC, N], f32, tag="ms")
                        ptr = ptr_full.rearrange("p b n -> p (b n)")[:, 0:PD]
                        nc.tensor.transpose(ptr[:], OFf[:, blk * 128:(blk + 1) * 128],
                                            ident[0:PD, 0:PD])
                        nc.scalar.copy(osb[:, blk, :], ptr[:])
                    # out rows (blk*128 + p), iterate (p, blk, pd)
                    row0 = bc * 384
                    oap = out_d[row0:row0 + 384, :].rearrange(
                        "(blk p) c -> p blk c", blk=3, p=128)
                    nc.sync.dma_start(oap, osb[:])

                LOOPN = int(os.environ.get("KERNEL_LOOP", "1"))
                if LOOPN > 1:
                    with tc.For_i(0, LOOPN, 1):
                        _phase_c()
                else:
                    _phase_c()

    nc.compile()
    return nc


def _build_and_run(host, in_maps, trace):
    from concourse.bass_utils import run_bass_kernel_spmd

    nc = _build(host)
    res = run_bass_kernel_spmd(
        nc, in_maps, core_ids=list(range(NCORES)), trace=trace,
        trace_cores=list(range(NCORES)) if trace else None,
        stitch_traces=bool(trace and NCORES > 1))
    return res


def _host_prep(**inputs):
    import ml_dtypes
    f8 = ml_dtypes.float8_e4m3
    bf16 = ml_dtypes.bfloat16

    x = _f32(inputs["x"])
    beta = _f32(inputs["beta"])
    context = _f32(inputs["context"])
    g = {k: np.asarray(v, dtype=np.float64) for k, v in inputs.items()
         if k not in ("x", "beta", "context")}

    # --- algebraic folds (host, tiny) ---
    embW = g["emb_w"][:, :, 0]            # [64, 3]
    dembW = g["demb_w"][:, :, 0]          # [3, 64]
    M3 = dembW @ embW                     # [3, 3]
    v3 = dembW @ g["emb_b"] + g["demb_b"]
    s3 = M3.sum(axis=1)

    pe = _pe_table().astype(np.float64)   # [N, F]

    c1aug = np.empty((3, F), np.float32)
    c1aug[0:2] = g["c1_w"].T * SX
    c1aug[2] = g["c1_b"] * SX

    # gate weights [C, 29*128]; c1 hyper rows pre-scaled by SX
    wg = np.zeros((C, 29 * 128), np.float32)
    wg[:, 0:1024] = g["c1_gw"].T
    wg[:, 1024:2048] = g["c1_hw"].T * SX
    wg[:, 2048:2560] = g["c3_gw"].T
    wg[:, 2560:3072] = g["c3_hw"].T
    wg[:, 3072:3328] = g["c4_gw"].T
    wg[:, 3328:3584] = g["c4_hw"].T
    wg[:, 3584:3586] = g["cl_gw"].T
    wg[:, 3586:3588] = g["cl_hw"].T
    gbias = np.zeros(29 * 128, np.float32)
    gbias[0:1024] = g["c1_gb"]
    gbias[2048:2560] = g["c3_gb"]
    gbias[3072:3328] = g["c4_gb"]
    gbias[3584:3586] = g["cl_gb"]
    gbias = np.ascontiguousarray(gbias.reshape(29, 128).T)
    # [c, p, k, o] = wg[k*128+p, c*128+o] for single-DMA per-chunk streaming
    wg = np.ascontiguousarray(
        wg.reshape(4, 128, 29, 128).transpose(2, 1, 0, 3)).astype(bf16)

    # conv weights -> [11, ci, co] tap-major with zero padding
    convt = np.zeros((11, F, F), np.float32)
    convt[5, :, 0:512] = g["conv1_w"][:, :, 0].T
    for t in range(3):
        convt[t + 4, :, 512:768] = g["conv2_w"][:, :, t].T
    for t in range(5):
        convt[t + 3, :, 768:832] = g["conv3_w"][:, :, t].T
    for t in range(7):
        convt[t + 2, :, 832:896] = g["conv4_w"][:, :, t].T
    for t in range(9):
        convt[t + 1, :, 896:960] = g["conv5_w"][:, :, t].T
    for t in range(11):
        convt[t, :, 960:1024] = g["conv6_w"][:, :, t].T
    # device layout: [ci_pair, p, blk, j, co_off] fp8 x SW
    convt_dev = np.empty((4, 128, NBLK, 2, 128), np.float32)
    for (co, d), idx in BLK.items():
        blkslab = (convt[d + 5, :, co * 128:(co + 1) * 128] * SW)  # [F, 128]
        convt_dev[:, :, idx, :, :] = blkslab.reshape(4, 2, 128, 128).transpose(
            0, 2, 1, 3)
    convt_dev = convt_dev.reshape(4, 128, NBLK * 2 * 128).astype(f8)

    # positional encoding + conv bias pushed through c3 (host, exact):
    peT = pe.T                             # [F, N] float64
    convt64 = convt.astype(np.float64)
    pe_conv = np.zeros((F, N), np.float64)
    for d in range(-5, 6):
        a, b2 = max(0, -d), N - max(0, d)
        pe_conv[:, a:b2] += convt64[d + 5].T @ peT[:, a + d:b2 + d]
    conv_bias = np.concatenate([g["conv1_b"], g["conv2_b"], g["conv3_b"],
                                g["conv4_b"], g["conv5_b"], g["conv6_b"]])
    c3bias = (g["c3_w"] @ (pe_conv + conv_bias[:, None])
              + g["c3_b"][:, None])                       # [C, N]
    # c3bT[n, m, q] = PS3 * c3bias[m*128+q, n]
    c3bT = np.ascontiguousarray(
        (c3bias.T * PS3).reshape(N, 4, 128)).astype(bf16)

    # c3 weights: c3w8[p, pc, j, q] = S3W * c3_w[q, (2pc+j)*128+p]
    c3w8 = np.ascontiguousarray(
        (g["c3_w"].T * S3W).reshape(4, 2, 128, C).transpose(2, 0, 1, 3)
    ).astype(f8)
    # c4 weights: c4w8[p, pc, j, q] = S4W * c4_w[q, (2pc+j)*128+p]
    c4w8 = np.ascontiguousarray(
        (g["c4_w"].T * S4W).reshape(2, 2, 128, 256).transpose(2, 0, 1, 3)
    ).astype(f8)
    # c4bx[q, m2] = PS4 * c4_b[m2*128+q]
    c4bx = _f32(PS4 * g["c4_b"].reshape(2, 128).T)
    clwt = _f32(g["cl_w"].T.reshape(2, 128, PD).transpose(1, 0, 2))
    clb = _f32(g["cl_b"].reshape(PD, 1))

    # indicator tiles: n-indicator [24, BC, N]; global-b indicator
    indn = np.zeros((24, BC, N), np.float32)
    for n_ in range(N):
        indn[n_, :, n_] = 1.0
    indn = indn.astype(bf16)
    indb = np.zeros((128, NBC, BC, N), np.float32)
    for bg in range(128):
        indb[bg, bg // BC, bg % BC, :] = 1.0
    indb = indb.reshape(128, NBC, BC * N).astype(f8)

    host = dict(M3=M3, v3=v3, s3=s3)

    sm32 = np.zeros((128, 32), np.float32)
    sm32[:, 0:29] = gbias
    sm32[:, 29:31] = c4bx
    sm32[0:PD, 31] = clb[:, 0]
    smbf = np.zeros((24, 896), np.float32)
    smbf[:, 0:512] = c3bT.astype(np.float32).reshape(24, 512)
    smbf[:, 512:896] = indn.astype(np.float32).reshape(24, 384)
    smbf = smbf.astype(bf16)
    shared = dict(wg=wg, sm32=sm32, smbf=smbf, convt=convt_dev,
                  c3w8=c3w8, c4w8=c4w8, clwt=clwt, indb=indb)
    xt_all = x.transpose(2, 0, 1).reshape(PD, B * N)   # [2, B*N]
    in_maps = []
    for k in range(NCORES):
        sl = slice(k * BLOC, (k + 1) * BLOC)
        xtc = np.empty((NBC, 3, F + FREE), np.float32)
        xtc[:, :, :F] = c1aug[None]
        for bc in range(NBC):
            lo = k * BLOC * N + bc * FREE
            xtc[bc, 0:2, F:] = xt_all[:, lo:lo + FREE]
            xtc[bc, 2, F:] = 1.0
        m = dict(shared)
        m["ctx"] = np.ascontiguousarray(context[sl])
        bl = beta[sl].astype(np.float64)
        m["betav"] = _f32(np.stack([bl, np.sin(bl), np.cos(bl)], axis=1))
        m["xtc"] = xtc
        in_maps.append(m)

    return host, in_maps


_LAST_HOST = None


def kernel(**inputs):
    global LAST_RESULTS, _LAST_HOST
    host, in_maps = _host_prep(**inputs)
    _LAST_HOST = host
    trace = bool(int(os.environ.get("KERNEL_TRACE", "0")))
    res = _build_and_run(host, in_maps, trace)
    LAST_RESULTS = res
    out = np.concatenate(
        [res.results[k]["out"].reshape(BLOC, N, PD) for k in range(NCORES)],
        axis=0)
    return out


# revision 42
# speedup vs baseline: 1.1207x; 1.1207x over previous
"""Trainium2 Bass kernel for nn_CNNConcatLinear (B=1024, N=24, PD=2, C=512).

Strategy: pure data-parallel over batch (128 per core x 8 cores).
Per core, channels-on-partitions layouts:

  phase A: new_ctx = context + (sum_j e_j t_j)/(sum_j e_j)  (the two 1x1
           convs + softmax collapse into a 3x3 system folded on host).
  phase B: all ConcatSquashLinear gates/hyper-biases as [feature, batch]
           bf16 matmuls from new_ctx^T.  Hyper-bias/gate ratios (h/g) are
           transposed once per core so later layers can fold "+ h" into
           their matmuls via batch-indicator contraction rows.
  phase C: pipeline over batch chunks of 16:
           c1 (x-rows + b-indicator fold rows) -> gated X in fp8 (x16) ->
           6 convs as fp8 DoubleRow matmuls (2 ci-chunks per pass) ->
           c3/c4 as fp8 DoubleRow with bias+hyper folded as extra
           matmul rows (n-indicator rows carry c3bias incl. positional
           encoding pushed through the convs on host) -> cl in f32r.

All epilogues are a single DVE op: out = (psum * 2^-k) * gate.
"""

import math
import os

import numpy as np

B, N, PD, C = 1024, 24, 2, 512
F = 2 * C
NCORES = 8
BLOC = B // NCORES          # 128 batch per core
BC = 16                     # batch chunk
NBC = BLOC // BC            # 8 chunks
PADL = 5
NW = N + 2 * PADL           # 34 (padded X width)
FREE = BC * N               # 384

# quantization scales (powers of 2)
SX = 16.0                   # X fp8 scale
SW = 128.0                  # conv weight fp8 scale
SY = 16.0                   # Y fp8 scale
S3W = 128.0                 # c3 weight fp8 scale
ST3 = 16.0                  # T3 fp8 scale
S4W = 128.0                 # c4 weight fp8 scale
SGW = 64.0                  # gate weight fp8 scale
SXR = 8.0                   # x input fp8 scale
IND = 176.0                 # indicator value (exact in e4m3)
SCX = 16.0                  # new_ctx fp8 scale
PS3 = SY * S3W              # c3 psum scale (2048)
PS4 = ST3 * S4W             # c4 psum scale (2048)

TAPS = {
    0: [0], 1: [0], 2: [0], 3: [0],
    4: [0, -1, 1], 5: [0, -1, 1],
    6: [0, -1, 1, -2, 2, -3, 3],
    7: [0, -1, 1, -2, 2, -3, 3, -4, 4, -5, 5],
}
BLK = {}
for _co in range(8):
    for _d in TAPS[_co]:
        BLK[(_co, _d)] = len(BLK)
NBLK = len(BLK)             # 28

LAST_RESULTS = None         # BassKernelResults from the most recent run


def _pe_table():
    pos = np.arange(N, dtype=np.float32)[:, None]
    div = np.exp(np.arange(0, F, 2, dtype=np.float32) * (-np.log(10000.0) / F))
    pe = np.zeros((N, F), dtype=np.float32)
    pe[:, 0::2] = np.sin(pos * div)
    pe[:, 1::2] = np.cos(pos * div)
    return pe


def _f32(a):
    return np.ascontiguousarray(np.asarray(a, dtype=np.float32))


def _build(host, num_devices=NCORES):
    import concourse.bass as bass
    import concourse.mybir as mybir
    import concourse.tile as tile
    from concourse import bacc
    from concourse.masks import make_identity

    f32 = mybir.dt.float32
    F8 = mybir.dt.float8e4
    BF = mybir.dt.bfloat16
    DT = mybir.dt.float32r
    DR = mybir.MatmulPerfMode.DoubleRow
    AluOp = mybir.AluOpType
    Act = mybir.ActivationFunctionType

    M3, v3, s3 = host["M3"], host["v3"], host["s3"]

    nc = bacc.Bacc("TRN2", target_bir_lowering=False, debug=False,
                   num_devices=num_devices)

    def din(name, shape, dt=f32):
        return nc.dram_tensor(name, list(shape), dt, kind="ExternalInput").ap()

    ctx_d = din("ctx", [BLOC, C])
    beta_d = din("betav", [BLOC, 3])
    xtc_d = din("xtc", [NBC, 3, FREE], F8)
    wg_d = din("wg", [29, 128, 4, 128], BF)
    sm32_d = din("sm32", [128, 32])
    c1ch_d = din("c1ch", [128, 8, 128], F8)
    c3f0_d = din("c3f0", [128, 4, 128], F8)
    convt_d = din("convt", [4, 128, NBLK * 2 * 128], F8)
    c3w8_d = din("c3w8", [128, 4, 2, C], F8)
    c4w8_d = din("c4w8", [128, 2, 2, 256], F8)
    clwt_d = din("clwt", [128, 2, PD], DT)
    indc_d = din("indc", [128, 2, NBC, BC * N], F8)
    out_d = nc.dram_tensor("out", [BLOC * N, PD], f32, kind="ExternalOutput").ap()
    DEBUG = bool(int(os.environ.get("KERNEL_DEBUG", "0")))
    if DEBUG:
        dbg_nctx = nc.dram_tensor("dbg_nctx", [128, C], f32, kind="ExternalOutput").ap()
        dbg_g = nc.dram_tensor("dbg_g", [128, 2048], f32, kind="ExternalOutput").ap()
        dbg_y = nc.dram_tensor("dbg_y", [8, 128, BC, N], f32, kind="ExternalOutput").ap()
        dbg_t3 = nc.dram_tensor("dbg_t3", [4, 128, BC, N], f32, kind="ExternalOutput").ap()

    with tile.TileContext(nc) as tc:
        import contextlib
        est = contextlib.ExitStack()
        with est:
            wp = est.enter_context(tc.tile_pool(name="wp", bufs=1))
            gout = est.enter_context(tc.tile_pool(name="gout", bufs=1))

            # ---------- persistent small tiles + their DMAs ----------
            beta_t0 = wp.tile([128, 3], f32, tag="beta0")
            nc.sync.dma_start(beta_t0[:], beta_d[:])
            ctx_t0 = wp.tile([128, C], f32, tag="ctx0")
            nc.sync.dma_start(ctx_t0[:], ctx_d[:])
            # merged small tensors
            sm32_s = wp.tile([128, 32], f32, tag="sm32")
            nc.sync.dma_start(sm32_s[:], sm32_d[:])
            gbias_s = sm32_s[:, 0:29]
            c4bx_s = sm32_s[:, 29:31]
            clb_s = sm32_s[0:PD, 31:32]
            ident = wp.tile([128, 128], f32, tag="ident")
            make_identity(nc, ident[:])
            # probes: stall the Pool/ACT DGE streams until ctx+beta landed so
            # their bulk transfers can't jump ahead on the DMA engines
            probe = wp.tile([1, 2], f32, tag="probe")
            nc.gpsimd.tensor_copy(probe[0:1, 0:1], ctx_t0[0:1, 0:1])
            nc.scalar.copy(probe[0:1, 1:2], ctx_t0[0:1, 0:1])
            indc_s = wp.tile([128, 2, NBC, BC * N], F8, tag="indc")
            xind_s = wp.tile([128, 2, NBC, BC * N], F8, tag="xind")
            nc.vector.memset(xind_s[:, 0], 0.0)
            c1comb_s = wp.tile([128, 2, 8, 128], F8, tag="c1comb")
            c3fold_s = wp.tile([128, 2, 4, 128], F8, tag="c3fold")
            c4fold_s = wp.tile([128, 2, 2, 128], F8, tag="c4fold")
            wg_all = wp.tile([128, 29, 4, 128], BF, tag="wg")
            _wb = [0, 8, 16, 24, 29]
            for _cw in range(4):
                _clo, _chi = _wb[_cw], _wb[_cw + 1]
                nc.scalar.dma_start(
                    wg_all[:, _clo:_chi],
                    wg_d[_clo:_chi].rearrange("c p k o -> p c k o"))
            nc.sync.dma_start(xind_s[0:3, 0, 0, :],
                              xtc_d[0].rearrange("p f -> p f"))

            # gate/hyper output tiles [feature_part, chunk*128 + b]
            g1b1_s = gout.tile([128, 2048], f32, tag="g1b1")
            g3h3_s = gout.tile([128, 1024], f32, tag="g3h3")
            g4h4_s = gout.tile([128, 512], f32, tag="g4h4")
            gl_s = gout.tile([PD, 128], f32, tag="gl")
            hl_s = gout.tile([PD, 128], f32, tag="hl")

            # ---------- heavy weight tiles ----------
            convw_s = []
            for pc in range(4):
                t = wp.tile([128, NBLK, 2, 128], F8, tag=f"convw{pc}")
                convw_s.append(t)
            c3w_all = wp.tile([128, 4, 2, C], F8, tag="c3w")
            c4w_all = wp.tile([128, 2, 2, 256], F8, tag="c4w")
            clw_all = wp.tile([128, 2, PD], DT, tag="clw")

            def emit_bulk_dmas():
                nc.gpsimd.dma_start(indc_s[:], indc_d[:])
                nc.gpsimd.dma_start(xind_s[:, 1], indc_d[:, 1])
                nc.gpsimd.dma_start(c1comb_s[:, 0], c1ch_d[:])
                nc.gpsimd.dma_start(c3fold_s[:, 0], c3f0_d[:])
                for pc in range(4):
                    nc.gpsimd.dma_start(
                        convw_s[pc][:],
                        convt_d[pc].rearrange(
                            "p (blk j o) -> p blk j o", blk=NBLK, j=2, o=128))
                for _bc in range(1, NBC):
                    nc.gpsimd.dma_start(xind_s[0:3, 0, _bc, :], xtc_d[_bc])
                nc.gpsimd.dma_start(c3w_all[:], c3w8_d[:])
                nc.gpsimd.dma_start(c4w_all[:], c4w8_d[:])
                nc.gpsimd.dma_start(clw_all[:], clwt_d[:])

            # ---------- phase-C SBUF pools (open early so c1(0) can run in B)
            xp = est.enter_context(tc.tile_pool(name="xp", bufs=1))

            def bcast(ap_2d, np_=N):
                return ap_2d.unsqueeze(2).broadcast_to(
                    [ap_2d.shape[0], BC, np_])

            nc.vector.memset(c4fold_s[:, 0], 0.0)
            X_gen = []
            for gen in range(2):
                gtiles = []
                for pc in range(4):
                    Xf = xp.tile([128, 2, BC, NW], F8, tag=f"x{gen}_{pc}")
                    gtiles.append(Xf)
                X_gen.append(gtiles)

            def _c1(bc, pspool, pstag="c1", psbufs=None):
                cs = bc * BC
                X_t = X_gen[bc % 2]
                for fc in range(8):
                    ps1 = pspool.tile([128, BC, N], f32, tag=pstag,
                                      bufs=psbufs)
                    nc.tensor.matmul(
                        ps1[:], c1comb_s[:, :, fc, :],
                        xind_s[:, :, bc, :].rearrange(
                            "p j (b n) -> p j b n", b=BC),
                        start=True, stop=True, perf_mode=DR)
                    xi = X_t[fc // 2][:, fc % 2, :, PADL:PADL + N]
                    g1 = bcast(g1b1_s[:, fc * 128 + cs:fc * 128 + cs + BC])
                    nc.vector.scalar_tensor_tensor(xi, ps1[:], 1.0 / SXR, g1,
                                                   AluOp.mult, AluOp.mult)

            # ---------- phase A: new_ctx (pipelined in 4 col blocks) ----
            with tc.tile_pool(name="pa", bufs=1) as pap:
                ctx_t = ctx_t0
                bvec = beta_t0          # [128, 3] = (beta, sin b, cos b)

                u = pap.tile([128, 3], f32, tag="u")
                for j in range(3):
                    uj = u[:, j:j + 1]
                    nc.vector.tensor_scalar(uj, bvec[:, 0:1], float(M3[j, 0]),
                                            float(v3[j]), AluOp.mult, AluOp.add)
                    nc.vector.scalar_tensor_tensor(uj, bvec[:, 1:2],
                                                   float(M3[j, 1]),
                                                   uj, AluOp.mult, AluOp.add)
                    nc.vector.scalar_tensor_tensor(uj, bvec[:, 2:3],
                                                   float(M3[j, 2]),
                                                   uj, AluOp.mult, AluOp.add)

                emit_bulk_dmas()
                psa_cm = tc.tile_pool(name="ps_a", bufs=1, space="PSUM")
                psa = psa_cm.__enter__()
                nctxT = wp.tile([128, C], BF, tag="nctxT")
                for kb in range(4):
                    sl = slice(kb * 128, (kb + 1) * 128)
                    z = pap.tile([128, 128], f32, tag="z", bufs=2)
                    num = pap.tile([128, 128], f32, tag="num", bufs=2)
                    for j in range(3):
                        ej = psa.tile([128, 128], f32, tag="ej", bufs=2)
                        nc.scalar.activation(ej[:], ctx_t[:, sl], Act.Exp,
                                             bias=u[:, j:j + 1],
                                             scale=float(s3[j]))
                        if j == 0:
                            nc.vector.tensor_copy(z[:], ej[:])
                            nc.vector.tensor_scalar(num[:], ej[:],
                                                    bvec[:, 0:1], None,
                                                    AluOp.mult)
                        else:
                            nc.vector.tensor_add(z[:], z[:], ej[:])
                            nc.vector.scalar_tensor_tensor(num[:], ej[:],
                                                           bvec[:, j:j + 1],
                                                           num[:], AluOp.mult,
                                                           AluOp.add)
                    nc.vector.reciprocal(z[:], z[:])
                    nc.vector.tensor_mul(num[:], num[:], z[:])
                    nc.vector.tensor_add(ctx_t[:, sl], ctx_t[:, sl], num[:])
                    pst = psa.tile([128, 128], f32, tag="tr", bufs=2)
                    nc.tensor.transpose(pst[:], ctx_t[:, sl], ident[:])
                    nc.scalar.copy(nctxT[:, sl], pst[:])
                nctx = ctx_t
                if DEBUG:
                    nc.sync.dma_start(dbg_nctx[:], nctx[:])
                psa_cm.__exit__(None, None, None)

            # X pad memsets (DVE is idle during the gate matmuls)
            for gen in range(2):
                for pc in range(4):
                    Xf = X_gen[gen][pc]
                    nc.vector.memset(Xf[:, :, :, 0:PADL], 0.0)
                    nc.vector.memset(Xf[:, :, :, PADL + N:NW], 0.0)

            # ---------- phase B: gates ----------
            with (
                tc.tile_pool(name="ps_b", bufs=8, space="PSUM") as pbp,
                tc.tile_pool(name="gprep", bufs=1) as gpp,
            ):
                def gdst(c):
                    # c1 hyper rows (c 8..15) are pre-scaled by SX on host
                    if c < 8:
                        return g1b1_s[:, c * 128:(c + 1) * 128], True
                    if c < 16:
                        return g1b1_s[:, 1024 + (c - 8) * 128:1024 + (c - 7) * 128], False
                    if c < 20:
                        return g3h3_s[:, (c - 16) * 128:(c - 15) * 128], True
                    if c < 24:
                        return g3h3_s[:, 512 + (c - 20) * 128:512 + (c - 19) * 128], False
                    if c < 26:
                        return g4h4_s[:, (c - 24) * 128:(c - 23) * 128], True
                    return g4h4_s[:, 256 + (c - 26) * 128:256 + (c - 25) * 128], False

                def fold_one(src_g, src_h, psc_scale, dstT, m, extra=None):
                    # dstT[:, m, :] = transpose of psc_scale * src_h / src_g
                    # (one 128-col chunk; spread across engines)
                    r_t = gpp.tile([128, 128], f32, tag="r1", bufs=4)
                    nc.vector.reciprocal(r_t[:], src_g)
                    hg = gpp.tile([128, 128], f32, tag="hg", bufs=4)
                    nc.vector.scalar_tensor_tensor(hg[:], src_h, psc_scale,
                                                   r_t[:], AluOp.mult,
                                                   AluOp.mult)
                    if extra is not None:
                        nc.vector.tensor_scalar(hg[:], hg[:], extra, None,
                                                AluOp.add)
                    pst = pbp.tile([128, 128], f32, tag="gtr", bufs=2)
                    nc.tensor.transpose(pst[:], hg[:], ident[:])
                    nc.vector.tensor_copy(dstT[:, m, :], pst[:])

                # one psum bank per gate chunk so accumulation groups of
                # consecutive chunks never share has_written state
                for c in range(29):
                    gp_bank = pbp.tile([128, 2, 128], f32, tag="gps", bufs=4)
                    gw_t = wg_all[:, c]
                    for k in range(4):
                        rhs = nctxT[:, k * 128:(k + 1) * 128]
                        if c == 28:
                            nc.tensor.matmul(gp_bank[0:2, 0, :],
                                             gw_t[:, k, 0:2], rhs,
                                             start=(k == 0), stop=False)
                            nc.tensor.matmul(gp_bank[0:2, 1, :],
                                             gw_t[:, k, 2:4], rhs,
                                             start=False, stop=(k == 3))
                        else:
                            nc.tensor.matmul(gp_bank[:, 0, :],
                                             gw_t[:, k], rhs,
                                             start=(k == 0), stop=(k == 3))
                    if c == 28:
                        nc.scalar.activation(gl_s[:], gp_bank[0:2, 0, :],
                                             Act.Sigmoid,
                                             bias=gbias_s[0:2, 28:29])
                        nc.scalar.copy(hl_s[:], gp_bank[0:2, 1, :])
                    else:
                        dst, is_g = gdst(c)
                        if is_g:
                            nc.scalar.activation(dst, gp_bank[:, 0, :],
                                                 Act.Sigmoid,
                                                 bias=gbias_s[:, c:c + 1])
                        else:
                            nc.scalar.copy(dst, gp_bank[:, 0, :])
                    # incremental hyper/gate fold transposes
                    if 8 <= c < 16:
                        fc = c - 8
                        fold_one(g1b1_s[:, fc * 128:(fc + 1) * 128],
                                 g1b1_s[:, 1024 + fc * 128:1024 + (fc + 1) * 128],
                                 SXR / IND, c1comb_s[:, 1], fc)
                    elif 20 <= c < 24:
                        m = c - 20
                        fold_one(g3h3_s[:, m * 128:(m + 1) * 128],
                                 g3h3_s[:, 512 + m * 128:512 + (m + 1) * 128],
                                 PS3 / IND, c3fold_s[:, 1], m)
                    elif 26 <= c < 28:
                        m2 = c - 26
                        fold_one(g4h4_s[:, m2 * 128:(m2 + 1) * 128],
                                 g4h4_s[:, 256 + m2 * 128:256 + (m2 + 1) * 128],
                                 PS4 / IND, c4fold_s[:, 1], m2,
                                 extra=c4bx_s[:, m2:m2 + 1])
                    if c == 15:
                        _c1(0, pbp, "gps2", psbufs=2)

                if DEBUG:
                    nc.sync.dma_start(dbg_g[:], g1b1_s[:])

            clw_s = [clw_all[:, k, :] for k in range(2)]

            # ---------- phase C ----------
            with (
                tc.tile_pool(name="yp", bufs=8) as yp,
                tc.tile_pool(name="t3p", bufs=4) as t3p,
                tc.tile_pool(name="obp", bufs=3) as obp,
                tc.tile_pool(name="ps_c1", bufs=1, space="PSUM") as ps_c1,
                tc.tile_pool(name="ps_cv", bufs=2, space="PSUM") as ps_cv,
                tc.tile_pool(name="ps_c3", bufs=2, space="PSUM") as ps_c3,
                tc.tile_pool(name="ps_ms", bufs=3, space="PSUM") as ps_ms,
            ):
                def _phase_c():
                  for bc in range(NBC):
                    cs = bc * BC
                    X_t = X_gen[bc % 2]

                    # --- convs (x16 fp8; bias + pe folded into c3bT)
                    Y_t = []
                    for co in range(0, 8, 2):
                        if co == 4 and bc + 1 < NBC:
                            _c1(bc + 1, ps_c1)
                        Yc = yp.tile([128, 2, BC, N], F8, tag="y")
                        for j in range(2):
                            cj = co + j
                            psc = ps_cv.tile([128, BC, N], f32, tag="conv")
                            mms = [(d, pc) for d in TAPS[cj] for pc in range(4)]
                            for i, (d, pc) in enumerate(mms):
                                nc.tensor.matmul(
                                    psc[:],
                                    convw_s[pc][:, BLK[(cj, d)], :, :],
                                    X_t[pc][:, :, :, PADL + d:PADL + d + N],
                                    start=(i == 0), stop=(i == len(mms) - 1),
                                    perf_mode=DR)
                            nc.scalar.mul(Yc[:, j], psc[:], SY / (SX * SW))
                            if DEBUG and bc == 0:
                                nc.sync.dma_start(dbg_y[cj], psc[:])
                        Y_t.append(Yc)

                    # --- c3 (DR over 4 Y pairs + bias/hyper fold rows)
                    T3_t = []
                    for m2 in range(2):
                        T3m = t3p.tile([128, 2, BC, N], F8, tag="t3")
                        for jm in range(2):
                            m = m2 * 2 + jm
                            ps3 = ps_c3.tile([128, BC, N], f32, tag="c3")
                            for pc in range(4):
                                nc.tensor.matmul(
                                    ps3[:],
                                    c3w_all[:, pc, :, m * 128:(m + 1) * 128],
                                    Y_t[pc][:], start=(pc == 0), stop=False,
                                    perf_mode=DR)
                            nc.tensor.matmul(
                                ps3[:], c3fold_s[:, :, m, :],
                                indc_s[:, :, bc, :].rearrange(
                                    "p j (b n) -> p j b n", b=BC),
                                start=False, stop=True, perf_mode=DR)
                            g3 = bcast(g3h3_s[:, m * 128 + cs:m * 128 + cs + BC])
                            nc.vector.scalar_tensor_tensor(
                                T3m[:, jm], ps3[:], ST3 / PS3, g3,
                                AluOp.mult, AluOp.mult)
                            if DEBUG and bc == 0:
                                nc.sync.dma_start(dbg_t3[m], ps3[:])
                        T3_t.append(T3m)

                    # --- c4 (DR over 2 T3 pairs + hyper fold rows)
                    T4_t = []
                    for m2 in range(2):
                        ps4 = ps_ms.tile([128, BC, N], f32, tag="ms")
                        for pc in range(2):
                            nc.tensor.matmul(
                                ps4[:],
                                c4w_all[:, pc, :, m2 * 128:(m2 + 1) * 128],
                                T3_t[pc][:], start=(pc == 0), stop=False,
                                perf_mode=DR)
                        nc.tensor.matmul(
                            ps4[:], c4fold_s[:, :, m2, :],
                            indc_s[:, :, bc, :].rearrange(
                                "p j (b n) -> p j b n", b=BC),
                            start=False, stop=True, perf_mode=DR)
                        T4m = t3p.tile([128, BC, N], DT, tag="t4")
                        g4 = bcast(g4h4_s[:, m2 * 128 + cs:m2 * 128 + cs + BC])
                        nc.vector.scalar_tensor_tensor(
                            T4m[:], ps4[:], 1.0 / PS4, g4,
                            AluOp.mult, AluOp.mult)
                        T4_t.append(T4m)

                    # --- cl
                    psl_full = ps_ms.tile([128, BC, N], f32, tag="ms")
                    psl = psl_full[0:PD]
                    for k in range(2):
                        nc.tensor.matmul(psl[:], clw_s[k], T4_t[k][:],
                                         start=(k == 0), stop=(k == 1))
                    OF_full = t3p.tile([128, BC, N], f32, tag="of")
                    OF = OF_full[0:PD]
                    gl = gl_s[:, cs:cs + BC].unsqueeze(2).broadcast_to([PD, BC, N])
                    hl = hl_s[:, cs:cs + BC].unsqueeze(2).broadcast_to([PD, BC, N])
                    nc.vector.scalar_tensor_tensor(OF[:], psl[:], clb_s[:], gl,
                                                   AluOp.add, AluOp.mult)
                    nc.vector.tensor_add(OF[:], OF[:], hl)

                    # --- transpose [2, 384] -> [384, 2] in 128-blocks, DMA out
                    OFf = OF[:].rearrange("p b n -> p (b n)")
                    osb = obp.tile([128, 3, PD], f32, tag="ob")
                    for blk in range(3):
                        ptr_full = ps_ms.tile([128, BC, N], f32, tag="ms")
                        ptr = ptr_full.rearrange("p b n -> p (b n)")[:, 0:PD]
                        nc.tensor.transpose(ptr[:], OFf[:, blk * 128:(blk + 1) * 128],
                                            ident[0:PD, 0:PD])
                        nc.scalar.copy(osb[:, blk, :], ptr[:])
                    # out rows (blk*128 + p), iterate (p, blk, pd)
                    row0 = bc * 384
                    oap = out_d[row0:row0 + 384, :].rearrange(
                        "(blk p) c -> p blk c", blk=3, p=128)
                    nc.sync.dma_start(oap, osb[:])

                LOOPN = int(os.environ.get("KERNEL_LOOP", "1"))
                if LOOPN > 1:
                    with tc.For_i(0, LOOPN, 1):
                        _phase_c()
                else:
                    _phase_c()

    nc.compile()
    return nc


def _build_and_run(host, in_maps, trace):
    from concourse.bass_utils import run_bass_kernel_spmd

    nc = _build(host)
    res = run_bass_kernel_spmd(
        nc, in_maps, core_ids=list(range(NCORES)), trace=trace,
        trace_cores=list(range(NCORES)) if trace else None,
        stitch_traces=bool(trace and NCORES > 1))
    return res


def _host_prep(**inputs):
    import ml_dtypes
    f8 = ml_dtypes.float8_e4m3
    bf16 = ml_dtypes.bfloat16

    x = _f32(inputs["x"])
    beta = _f32(inputs["beta"])
    context = _f32(inputs["context"])
    g = {k: np.asarray(v, dtype=np.float64) for k, v in inputs.items()
         if k not in ("x", "beta", "context")}

    # --- algebraic folds (host, tiny) ---
    embW = g["emb_w"][:, :, 0]            # [64, 3]
    dembW = g["demb_w"][:, :, 0]          # [3, 64]
    M3 = dembW @ embW                     # [3, 3]
    v3 = dembW @ g["emb_b"] + g["demb_b"]
    s3 = M3.sum(axis=1)

    pe = _pe_table().astype(np.float64)   # [N, F]

    c1ch = np.zeros((128, 8, 128), np.float32)
    c1aug = np.empty((3, F), np.float64)
    c1aug[0:2] = g["c1_w"].T * SX
    c1aug[2] = g["c1_b"] * SX
    c1ch[0:3] = c1aug.reshape(3, 8, 128)
    c1ch = c1ch.astype(f8)

    # gate weights [C, 29*128]; c1 hyper rows pre-scaled by SX
    wg = np.zeros((C, 29 * 128), np.float32)
    wg[:, 0:1024] = g["c1_gw"].T
    wg[:, 1024:2048] = g["c1_hw"].T * SX
    wg[:, 2048:2560] = g["c3_gw"].T
    wg[:, 2560:3072] = g["c3_hw"].T
    wg[:, 3072:3328] = g["c4_gw"].T
    wg[:, 3328:3584] = g["c4_hw"].T
    wg[:, 3584:3586] = g["cl_gw"].T
    wg[:, 3586:3588] = g["cl_hw"].T
    gbias = np.zeros(29 * 128, np.float32)
    gbias[0:1024] = g["c1_gb"]
    gbias[2048:2560] = g["c3_gb"]
    gbias[3072:3328] = g["c4_gb"]
    gbias[3584:3586] = g["cl_gb"]
    gbias = np.ascontiguousarray(gbias.reshape(29, 128).T)
    # [c, p, k, o] = wg[k*128+p, c*128+o] for single-DMA per-chunk streaming
    wg = np.ascontiguousarray(
        wg.reshape(4, 128, 29, 128).transpose(2, 1, 0, 3)).astype(bf16)

    # conv weights -> [11, ci, co] tap-major with zero padding
    convt = np.zeros((11, F, F), np.float32)
    convt[5, :, 0:512] = g["conv1_w"][:, :, 0].T
    for t in range(3):
        convt[t + 4, :, 512:768] = g["conv2_w"][:, :, t].T
    for t in range(5):
        convt[t + 3, :, 768:832] = g["conv3_w"][:, :, t].T
    for t in range(7):
        convt[t + 2, :, 832:896] = g["conv4_w"][:, :, t].T
    for t in range(9):
        convt[t + 1, :, 896:960] = g["conv5_w"][:, :, t].T
    for t in range(11):
        convt[t, :, 960:1024] = g["conv6_w"][:, :, t].T
    # device layout: [ci_pair, p, blk, j, co_off] fp8 x SW
    convt_dev = np.empty((4, 128, NBLK, 2, 128), np.float32)
    for (co, d), idx in BLK.items():
        blkslab = (convt[d + 5, :, co * 128:(co + 1) * 128] * SW)  # [F, 128]
        convt_dev[:, :, idx, :, :] = blkslab.reshape(4, 2, 128, 128).transpose(
            0, 2, 1, 3)
    convt_dev = convt_dev.reshape(4, 128, NBLK * 2 * 128).astype(f8)

    # positional encoding + conv bias pushed through c3 (host, exact):
    peT = pe.T                             # [F, N] float64
    convt64 = convt.astype(np.float64)
    pe_conv = np.zeros((F, N), np.float64)
    for d in range(-5, 6):
        a, b2 = max(0, -d), N - max(0, d)
        pe_conv[:, a:b2] += convt64[d + 5].T @ peT[:, a + d:b2 + d]
    conv_bias = np.concatenate([g["conv1_b"], g["conv2_b"], g["conv3_b"],
                                g["conv4_b"], g["conv5_b"], g["conv6_b"]])
    c3bias = (g["c3_w"] @ (pe_conv + conv_bias[:, None])
              + g["c3_b"][:, None])                       # [C, N]
    # c3f0[n, m, q] = (PS3/IND) * c3bias[m*128+q, n], zero padded to 128 rows
    c3f0 = np.zeros((128, 4, 128), np.float32)
    c3f0[0:N] = (c3bias.T * (PS3 / IND)).reshape(N, 4, 128)
    c3f0 = c3f0.astype(f8)

    # c3 weights: c3w8[p, pc, j, q] = S3W * c3_w[q, (2pc+j)*128+p]
    c3w8 = np.ascontiguousarray(
        (g["c3_w"].T * S3W).reshape(4, 2, 128, C).transpose(2, 0, 1, 3)
    ).astype(f8)
    # c4 weights: c4w8[p, pc, j, q] = S4W * c4_w[q, (2pc+j)*128+p]
    c4w8 = np.ascontiguousarray(
        (g["c4_w"].T * S4W).reshape(2, 2, 128, 256).transpose(2, 0, 1, 3)
    ).astype(f8)
    # c4bx[q, m2] = (PS4/IND) * c4_b[m2*128+q]
    c4bx = _f32((PS4 / IND) * g["c4_b"].reshape(2, 128).T)
    clwt = _f32(g["cl_w"].T.reshape(2, 128, PD).transpose(1, 0, 2))
    clb = _f32(g["cl_b"].reshape(PD, 1))

    # combined indicator: j0 = n-indicator, j1 = global-b indicator (x IND)
    indc = np.zeros((128, 2, NBC, BC, N), np.float32)
    for n_ in range(N):
        indc[n_, 0, :, :, n_] = IND
    for bg in range(128):
        indc[bg, 1, bg // BC, bg % BC, :] = IND
    indc = indc.reshape(128, 2, NBC, BC * N).astype(f8)

    host = dict(M3=M3, v3=v3, s3=s3)

    sm32 = np.zeros((128, 32), np.float32)
    sm32[:, 0:29] = gbias
    sm32[:, 29:31] = c4bx
    sm32[0:PD, 31] = clb[:, 0]
    shared = dict(wg=wg, sm32=sm32, convt=convt_dev, c1ch=c1ch, c3f0=c3f0,
                  c3w8=c3w8, c4w8=c4w8, clwt=clwt, indc=indc)
    xt_all = x.transpose(2, 0, 1).reshape(PD, B * N)   # [2, B*N]
    in_maps = []
    for k in range(NCORES):
        sl = slice(k * BLOC, (k + 1) * BLOC)
        xtc = np.empty((NBC, 3, FREE), np.float32)
        for bc in range(NBC):
            lo = k * BLOC * N + bc * FREE
            xtc[bc, 0:2] = xt_all[:, lo:lo + FREE] * SXR
            xtc[bc, 2] = SXR
        xtc = xtc.astype(f8)
        m = dict(shared)
        m["ctx"] = np.ascontiguousarray(context[sl])
        bl = beta[sl].astype(np.float64)
        m["betav"] = _f32(np.stack([bl, np.sin(bl), np.cos(bl)], axis=1))
        m["xtc"] = xtc
        in_maps.append(m)

    return host, in_maps


_LAST_HOST = None


def kernel(**inputs):
    global LAST_RESULTS, _LAST_HOST
    host, in_maps = _host_prep(**inputs)
    _LAST_HOST = host
    trace = bool(int(os.environ.get("KERNEL_TRACE", "0")))
    res = _build_and_run(host, in_maps, trace)
    LAST_RESULTS = res
    out = np.concatenate(
        [res.results[k]["out"].reshape(BLOC, N, PD) for k in range(NCORES)],
        axis=0)
    return out
